# revision 10
# baseline (speedup 1.0000x reference)
"""DotLoss kernel for Trainium2, data-parallel over 8 NeuronCores.

loss = mean_i[ relu(1 + dot(img[I[i]], aud[i]) - dot(img[i], aud[i]))
             + relu(1 + dot(img[i], aud[A[i]]) - dot(img[i], aud[i])) ]

Sharding strategy (per the problem's sharding hint): data-parallel over
the batch axis with impostor rows made LOCAL TO EACH SHARD — the host
materializes img[I[i]] / aud[A[i]] for each shard's rows while slicing
inputs, so every core consumes four aligned, contiguous streams and the
device kernel is pure streaming at HBM bandwidth: no SWDGE gathers, no
GPSIMD descriptor generation (a serial ~73us/core Q7-ucode chain in the
gather design). Local streams are bf16; impostor streams are fp8-e4m3
(halves their bytes; the hinge mean is insensitive to the extra
rounding). 12MB/core total, pre-blocked on host as [chunk][partition]
[contig 4KB] so every HWDGE descriptor is one fat contiguous segment.

Compute is split so every engine runs its fastest mode:
  - streams land D-MAJOR: SBUF tile [128, a=4, s] holds D-component
    a*128+p of row s at (partition p, slot a).
  - ScalarE: converts fp8 impostor tiles to bf16 (activation Copy), and
    computes the hinge: activation(Relu, scale=1, bias=1, accum_out)
    straight off PSUM — hinge + sum in one instruction.
  - DVE: plain tensor_tensor products (bf16 2x mode — the only DVE op
    class with a 2-elem/cycle uop; scalar_tensor_tensor+accum is stuck
    at 1x) — 3 big [128, 2048] multiplies per chunk, nothing else.
  - TensorE: the sum over D is a partition-axis reduction = matmul with
    a +/-ones stationary. PSUM X accumulates iimp-anchor directly
    (products(gi*la) @ +ones, products(li*la) @ -ones), PSUM Y
    aimp-anchor. The anchor subtraction costs no extra DVE work.
Each core emits a [128, 2*nchunks] fp32 partial tile reduced to [128,1]
(all partitions identical broadcast sums -> host reads row 0, sums the
8 cores, divides by N). Per-row triples stay aligned because all four
streams use the same (chunk, slot) mapping; the loss sum is
permutation-invariant.
"""

import numpy as np

N, D = 32768, 512
NCORES = 8
SHARD = N // NCORES          # 4096 rows per core
P = 128
A = D // P                   # 4 partition-blocks of D
CH = 512                     # rows per chunk
NCH = SHARD // CH            # 8 chunks
_CACHE = {}


def _build_nc():
    import concourse.bacc as bacc
    import concourse.mybir as mybir
    import concourse.tile as tile
    from contextlib import ExitStack

    fp32 = mybir.dt.float32
    bf16 = mybir.dt.bfloat16
    fp8 = mybir.dt.float8e4

    nc = bacc.Bacc("TRN2")
    # D-major, chunk-blocked streams: [NCH, P, A, CH]
    img_loc = nc.dram_tensor("img_loc", [NCH, P, A, CH], bf16,
                             kind="ExternalInput")
    aud_loc = nc.dram_tensor("aud_loc", [NCH, P, A, CH], bf16,
                             kind="ExternalInput")
    img_imp = nc.dram_tensor("img_imp", [NCH, P, A, CH], fp8,
                             kind="ExternalInput")
    aud_imp = nc.dram_tensor("aud_imp", [NCH, P, A, CH], fp8,
                             kind="ExternalInput")
    onesc = nc.dram_tensor("onesc", [P, 2 * P], bf16, kind="ExternalInput")
    partial = nc.dram_tensor("partial", [P, 1], fp32, kind="ExternalOutput")

    mult = mybir.AluOpType.mult
    add = mybir.AluOpType.add
    relu = mybir.ActivationFunctionType.Relu
    copyf = mybir.ActivationFunctionType.Copy

    with ExitStack() as ctx:
        tc = ctx.enter_context(tile.TileContext(nc))
        lio = ctx.enter_context(tc.tile_pool(name="lio", bufs=4))
        gio = ctx.enter_context(tc.tile_pool(name="gio", bufs=4))
        gcv = ctx.enter_context(tc.tile_pool(name="gcv", bufs=4))
        prp = ctx.enter_context(tc.tile_pool(name="prp", bufs=6))
        psp = ctx.enter_context(tc.psum_pool(name="psp", bufs=4))
        hxp = ctx.enter_context(tc.tile_pool(name="hxp", bufs=4))
        acc = ctx.enter_context(tc.tile_pool(name="acc", bufs=1))

        ones_sb = acc.tile([P, 2 * P], bf16, tag="ones")
        nc.sync.dma_start(out=ones_sb[:], in_=onesc[:])
        pos = ones_sb[:, 0:P]
        neg = ones_sb[:, P:2 * P]

        hsum = acc.tile([P, 2 * NCH], fp32, tag="hsum")

        for k in range(NCH):
            li = lio.tile([P, A, CH], bf16, tag="li")
            nc.sync.dma_start(out=li[:], in_=img_loc[k])
            la = lio.tile([P, A, CH], bf16, tag="la")
            nc.sync.dma_start(out=la[:], in_=aud_loc[k])
            gi8 = gio.tile([P, A, CH], fp8, tag="gi8")
            nc.sync.dma_start(out=gi8[:], in_=img_imp[k])
            ga8 = gio.tile([P, A, CH], fp8, tag="ga8")
            nc.sync.dma_start(out=ga8[:], in_=aud_imp[k])

            # fp8->bf16 converts on the otherwise-idle GPSIMD engine; ACT
            # (hinge) and DVE (products) have no spare cycles for them.
            gi = gcv.tile([P, A, CH], bf16, tag="gi")
            nc.gpsimd.tensor_copy(out=gi[:], in_=gi8[:])
            ga = gcv.tile([P, A, CH], bf16, tag="ga")
            nc.gpsimd.tensor_copy(out=ga[:], in_=ga8[:])

            prA = prp.tile([P, A, CH], bf16, tag="prA")
            nc.vector.tensor_tensor(out=prA[:], in0=li[:], in1=la[:], op=mult)
            prI = prp.tile([P, A, CH], bf16, tag="prI")
            nc.vector.tensor_tensor(out=prI[:], in0=gi[:], in1=la[:], op=mult)
            prU = prp.tile([P, A, CH], bf16, tag="prU")
            nc.vector.tensor_tensor(out=prU[:], in0=li[:], in1=ga[:], op=mult)

            px = psp.tile([P, CH], fp32, tag="px")
            for a in range(A):
                nc.tensor.matmul(px[:], pos, prI[:, a], start=(a == 0),
                                 stop=False)
            for a in range(A):
                nc.tensor.matmul(px[:], neg, prA[:, a], start=False,
                                 stop=(a == A - 1))
            py = psp.tile([P, CH], fp32, tag="py")
            for a in range(A):
                nc.tensor.matmul(py[:], pos, prU[:, a], start=(a == 0),
                                 stop=False)
            for a in range(A):
                nc.tensor.matmul(py[:], neg, prA[:, a], start=False,
                                 stop=(a == A - 1))

            hx = hxp.tile([P, CH], bf16, tag="hx")
            nc.scalar.activation(out=hx[:], in_=px[:], func=relu, bias=1.0,
                                 scale=1.0, accum_out=hsum[:, 2 * k:2 * k + 1])
            hy = hxp.tile([P, CH], bf16, tag="hy")
            nc.scalar.activation(out=hy[:], in_=py[:], func=relu, bias=1.0,
                                 scale=1.0,
                                 accum_out=hsum[:, 2 * k + 1:2 * k + 2])

        psum_t = acc.tile([P, 1], fp32, tag="psum")
        nc.vector.tensor_reduce(
            out=psum_t[:], in_=hsum[:], axis=mybir.AxisListType.X, op=add,
        )
        nc.sync.dma_start(out=partial[:], in_=psum_t[:])

    nc.compile()
    return nc


def _get_nc():
    if "nc" not in _CACHE:
        _CACHE["nc"] = _build_nc()
    return _CACHE["nc"]


def _block(xt):
    """[D, SHARD] -> [NCH, P, A, CH]: per (chunk, partition) contiguous."""
    return np.ascontiguousarray(
        xt.reshape(A, P, NCH, CH).transpose(2, 1, 0, 3))


def make_in_maps(image_outputs, audio_outputs, I_imp_ind, A_imp_ind):
    import ml_dtypes

    bf16 = np.dtype(ml_dtypes.bfloat16)
    fp8 = np.dtype(ml_dtypes.float8_e4m3fn)
    img = np.asarray(image_outputs, dtype=np.float32)
    aud = np.asarray(audio_outputs, dtype=np.float32)
    I_imp = np.asarray(I_imp_ind).astype(np.int64)
    A_imp = np.asarray(A_imp_ind).astype(np.int64)
    ones = np.concatenate(
        [np.ones((P, P), np.float32), -np.ones((P, P), np.float32)],
        axis=1).astype(bf16)
    in_maps = []
    for c in range(NCORES):
        base = c * SHARD
        sl = slice(base, base + SHARD)
        in_maps.append({
            "img_loc": _block(img[sl].T.astype(bf16)),
            "aud_loc": _block(aud[sl].T.astype(bf16)),
            "img_imp": _block(img[I_imp[sl]].T.astype(fp8)),
            "aud_imp": _block(aud[A_imp[sl]].T.astype(fp8)),
            "onesc": ones,
        })
    return in_maps


def kernel(image_outputs, audio_outputs, I_imp_ind, A_imp_ind):
    from concourse import bass_utils

    nc = _get_nc()
    in_maps = make_in_maps(image_outputs, audio_outputs, I_imp_ind, A_imp_ind)
    res = bass_utils.run_bass_kernel_spmd(nc, in_maps, list(range(NCORES))).results
    # every PSUM partition holds identical broadcast sums -> use row 0 only
    total = sum(float(r["partial"][0, 0]) for r in res)
    return np.float32(total / N)


# revision 11
# speedup vs baseline: 2.3116x; 2.3116x over previous
"""DotLoss kernel for Trainium2, data-parallel over 8 NeuronCores.

loss = mean_i[ relu(1 + dot(img[I[i]], aud[i]) - dot(img[i], aud[i]))
             + relu(1 + dot(img[i], aud[A[i]]) - dot(img[i], aud[i])) ]

Sharding strategy (per the problem's sharding hint): data-parallel over
the batch axis with impostor rows made LOCAL TO EACH SHARD — the host
materializes img[I[i]] / aud[A[i]] for each shard's rows while slicing
inputs, so every core consumes four aligned, contiguous streams and the
device kernel is pure streaming at HBM bandwidth: no SWDGE gathers, no
GPSIMD descriptor generation (a serial ~73us/core Q7-ucode chain in the
gather design). Local streams are bf16; impostor streams are fp8-e4m3
(halves their bytes; the hinge mean is insensitive to the extra
rounding). 12MB/core total, pre-blocked on host as [chunk][partition]
[contig 4KB] so every HWDGE descriptor is one fat contiguous segment.

Compute is split so every engine runs its fastest mode:
  - streams land D-MAJOR: SBUF tile [128, a=4, s] holds D-component
    a*128+p of row s at (partition p, slot a).
  - ScalarE: converts fp8 impostor tiles to bf16 (activation Copy), and
    computes the hinge: activation(Relu, scale=1, bias=1, accum_out)
    straight off PSUM — hinge + sum in one instruction.
  - DVE: plain tensor_tensor products (bf16 2x mode — the only DVE op
    class with a 2-elem/cycle uop; scalar_tensor_tensor+accum is stuck
    at 1x) — 3 big [128, 2048] multiplies per chunk, nothing else.
  - TensorE: the sum over D is a partition-axis reduction = matmul with
    a +/-ones stationary. PSUM X accumulates iimp-anchor directly
    (products(gi*la) @ +ones, products(li*la) @ -ones), PSUM Y
    aimp-anchor. The anchor subtraction costs no extra DVE work.
Each core emits a [128, 2*nchunks] fp32 partial tile reduced to [128,1]
(all partitions identical broadcast sums -> host reads row 0, sums the
8 cores, divides by N). Per-row triples stay aligned because all four
streams use the same (chunk, slot) mapping; the loss sum is
permutation-invariant.
"""

import numpy as np

N, D = 32768, 512
NCORES = 8
SHARD = N // NCORES          # 4096 rows per core
P = 128
A = D // P                   # 4 partition-blocks of D
CH = 512                     # rows per chunk
NCH = SHARD // CH            # 8 chunks
_CACHE = {}


def _build_nc():
    import concourse.bacc as bacc
    import concourse.mybir as mybir
    import concourse.tile as tile
    from contextlib import ExitStack

    fp32 = mybir.dt.float32
    bf16 = mybir.dt.bfloat16
    fp8 = mybir.dt.float8e4

    nc = bacc.Bacc("TRN2")
    # D-major, chunk-blocked streams: [NCH, P, A, CH]
    img_loc = nc.dram_tensor("img_loc", [NCH, P, A, CH], bf16,
                             kind="ExternalInput")
    aud_loc = nc.dram_tensor("aud_loc", [NCH, P, A, CH], bf16,
                             kind="ExternalInput")
    img_imp = nc.dram_tensor("img_imp", [NCH, P, A, CH], fp8,
                             kind="ExternalInput")
    aud_imp = nc.dram_tensor("aud_imp", [NCH, P, A, CH], fp8,
                             kind="ExternalInput")
    onesc = nc.dram_tensor("onesc", [P, 2 * P], bf16, kind="ExternalInput")
    partial = nc.dram_tensor("partial", [P, 1], fp32, kind="ExternalOutput")

    mult = mybir.AluOpType.mult
    add = mybir.AluOpType.add
    relu = mybir.ActivationFunctionType.Relu
    copyf = mybir.ActivationFunctionType.Copy

    with ExitStack() as ctx:
        tc = ctx.enter_context(tile.TileContext(nc))
        lio = ctx.enter_context(tc.tile_pool(name="lio", bufs=4))
        gio = ctx.enter_context(tc.tile_pool(name="gio", bufs=4))
        gcv = ctx.enter_context(tc.tile_pool(name="gcv", bufs=4))
        prp = ctx.enter_context(tc.tile_pool(name="prp", bufs=6))
        psp = ctx.enter_context(tc.psum_pool(name="psp", bufs=4))
        hxp = ctx.enter_context(tc.tile_pool(name="hxp", bufs=4))
        acc = ctx.enter_context(tc.tile_pool(name="acc", bufs=1))

        ones_sb = acc.tile([P, 2 * P], bf16, tag="ones")
        nc.sync.dma_start(out=ones_sb[:], in_=onesc[:])
        pos = ones_sb[:, 0:P]
        neg = ones_sb[:, P:2 * P]

        hsum = acc.tile([P, 2 * NCH], fp32, tag="hsum")

        for k in range(NCH):
            li = lio.tile([P, A, CH], bf16, tag="li")
            nc.sync.dma_start(out=li[:], in_=img_loc[k])
            la = lio.tile([P, A, CH], bf16, tag="la")
            nc.sync.dma_start(out=la[:], in_=aud_loc[k])
            gi8 = gio.tile([P, A, CH], fp8, tag="gi8")
            nc.sync.dma_start(out=gi8[:], in_=img_imp[k])
            ga8 = gio.tile([P, A, CH], fp8, tag="ga8")
            nc.sync.dma_start(out=ga8[:], in_=aud_imp[k])

            # fp8->bf16 converts on ScalarE (GPSIMD streams ~5x slower here)
            gi = gcv.tile([P, A, CH], bf16, tag="gi")
            nc.scalar.activation(out=gi[:], in_=gi8[:], func=copyf)
            ga = gcv.tile([P, A, CH], bf16, tag="ga")
            nc.scalar.activation(out=ga[:], in_=ga8[:], func=copyf)

            prA = prp.tile([P, A, CH], bf16, tag="prA")
            nc.vector.tensor_tensor(out=prA[:], in0=li[:], in1=la[:], op=mult)
            prI = prp.tile([P, A, CH], bf16, tag="prI")
            nc.vector.tensor_tensor(out=prI[:], in0=gi[:], in1=la[:], op=mult)
            prU = prp.tile([P, A, CH], bf16, tag="prU")
            nc.vector.tensor_tensor(out=prU[:], in0=li[:], in1=ga[:], op=mult)

            px = psp.tile([P, CH], fp32, tag="px")
            for a in range(A):
                nc.tensor.matmul(px[:], pos, prI[:, a], start=(a == 0),
                                 stop=False)
            for a in range(A):
                nc.tensor.matmul(px[:], neg, prA[:, a], start=False,
                                 stop=(a == A - 1))
            py = psp.tile([P, CH], fp32, tag="py")
            for a in range(A):
                nc.tensor.matmul(py[:], pos, prU[:, a], start=(a == 0),
                                 stop=False)
            for a in range(A):
                nc.tensor.matmul(py[:], neg, prA[:, a], start=False,
                                 stop=(a == A - 1))

            hx = hxp.tile([P, CH], bf16, tag="hx")
            nc.scalar.activation(out=hx[:], in_=px[:], func=relu, bias=1.0,
                                 scale=1.0, accum_out=hsum[:, 2 * k:2 * k + 1])
            hy = hxp.tile([P, CH], bf16, tag="hy")
            nc.scalar.activation(out=hy[:], in_=py[:], func=relu, bias=1.0,
                                 scale=1.0,
                                 accum_out=hsum[:, 2 * k + 1:2 * k + 2])

        psum_t = acc.tile([P, 1], fp32, tag="psum")
        nc.vector.tensor_reduce(
            out=psum_t[:], in_=hsum[:], axis=mybir.AxisListType.X, op=add,
        )
        nc.sync.dma_start(out=partial[:], in_=psum_t[:])

    nc.compile()
    return nc


def _get_nc():
    if "nc" not in _CACHE:
        _CACHE["nc"] = _build_nc()
    return _CACHE["nc"]


def _block(xt):
    """[D, SHARD] -> [NCH, P, A, CH]: per (chunk, partition) contiguous."""
    return np.ascontiguousarray(
        xt.reshape(A, P, NCH, CH).transpose(2, 1, 0, 3))


def make_in_maps(image_outputs, audio_outputs, I_imp_ind, A_imp_ind):
    import ml_dtypes

    bf16 = np.dtype(ml_dtypes.bfloat16)
    fp8 = np.dtype(ml_dtypes.float8_e4m3fn)
    img = np.asarray(image_outputs, dtype=np.float32)
    aud = np.asarray(audio_outputs, dtype=np.float32)
    I_imp = np.asarray(I_imp_ind).astype(np.int64)
    A_imp = np.asarray(A_imp_ind).astype(np.int64)
    ones = np.concatenate(
        [np.ones((P, P), np.float32), -np.ones((P, P), np.float32)],
        axis=1).astype(bf16)
    in_maps = []
    for c in range(NCORES):
        base = c * SHARD
        sl = slice(base, base + SHARD)
        in_maps.append({
            "img_loc": _block(img[sl].T.astype(bf16)),
            "aud_loc": _block(aud[sl].T.astype(bf16)),
            "img_imp": _block(img[I_imp[sl]].T.astype(fp8)),
            "aud_imp": _block(aud[A_imp[sl]].T.astype(fp8)),
            "onesc": ones,
        })
    return in_maps


def kernel(image_outputs, audio_outputs, I_imp_ind, A_imp_ind):
    from concourse import bass_utils

    nc = _get_nc()
    in_maps = make_in_maps(image_outputs, audio_outputs, I_imp_ind, A_imp_ind)
    res = bass_utils.run_bass_kernel_spmd(nc, in_maps, list(range(NCORES))).results
    # every PSUM partition holds identical broadcast sums -> use row 0 only
    total = sum(float(r["partial"][0, 0]) for r in res)
    return np.float32(total / N)


# revision 12
# speedup vs baseline: 2.4076x; 1.0415x over previous
"""DotLoss kernel for Trainium2, data-parallel over 8 NeuronCores.

loss = mean_i[ relu(1 + dot(img[I[i]], aud[i]) - dot(img[i], aud[i]))
             + relu(1 + dot(img[i], aud[A[i]]) - dot(img[i], aud[i])) ]

Sharding strategy (per the problem's sharding hint): data-parallel over
the batch axis with impostor rows made LOCAL TO EACH SHARD — the host
materializes img[I[i]] / aud[A[i]] for each shard's rows while slicing
inputs, so every core consumes four aligned, contiguous streams and the
device kernel is pure streaming at HBM bandwidth: no SWDGE gathers, no
GPSIMD descriptor generation (a serial ~73us/core Q7-ucode chain in the
gather design). Local streams are bf16; impostor streams are fp8-e4m3
(halves their bytes; the hinge mean is insensitive to the extra
rounding). 12MB/core total, pre-blocked on host as [chunk][partition]
[contig 4KB] so every HWDGE descriptor is one fat contiguous segment.

Compute is split so every engine runs its fastest mode:
  - streams land D-MAJOR: SBUF tile [128, a=4, s] holds D-component
    a*128+p of row s at (partition p, slot a).
  - ScalarE: converts fp8 impostor tiles to bf16 (activation Copy), and
    computes the hinge: activation(Relu, scale=1, bias=1, accum_out)
    straight off PSUM — hinge + sum in one instruction.
  - DVE: plain tensor_tensor products (bf16 2x mode — the only DVE op
    class with a 2-elem/cycle uop; scalar_tensor_tensor+accum is stuck
    at 1x) — 3 big [128, 2048] multiplies per chunk, nothing else.
  - TensorE: the sum over D is a partition-axis reduction = matmul with
    a +/-ones stationary. PSUM X accumulates iimp-anchor directly
    (products(gi*la) @ +ones, products(li*la) @ -ones), PSUM Y
    aimp-anchor. The anchor subtraction costs no extra DVE work.
Each core emits a [128, 2*nchunks] fp32 partial tile reduced to [128,1]
(all partitions identical broadcast sums -> host reads row 0, sums the
8 cores, divides by N). Per-row triples stay aligned because all four
streams use the same (chunk, slot) mapping; the loss sum is
permutation-invariant.
"""

import numpy as np

N, D = 32768, 512
NCORES = 8
SHARD = N // NCORES          # 4096 rows per core
P = 128
A = D // P                   # 4 partition-blocks of D
CH = 512                     # rows per chunk
NCH = SHARD // CH            # 8 chunks
_CACHE = {}


def _build_nc():
    import concourse.bacc as bacc
    import concourse.mybir as mybir
    import concourse.tile as tile
    from contextlib import ExitStack

    fp32 = mybir.dt.float32
    bf16 = mybir.dt.bfloat16
    fp8 = mybir.dt.float8e4

    nc = bacc.Bacc("TRN2")
    # D-major, chunk-blocked streams: [NCH, P, A, CH]
    img_loc = nc.dram_tensor("img_loc", [NCH, P, A, CH], bf16,
                             kind="ExternalInput")
    aud_loc = nc.dram_tensor("aud_loc", [NCH, P, A, CH], bf16,
                             kind="ExternalInput")
    img_imp = nc.dram_tensor("img_imp", [NCH, P, A, CH], bf16,
                             kind="ExternalInput")
    aud_imp = nc.dram_tensor("aud_imp", [NCH, P, A, CH], bf16,
                             kind="ExternalInput")
    onesc = nc.dram_tensor("onesc", [P, 2 * P], bf16, kind="ExternalInput")
    partial = nc.dram_tensor("partial", [P, 1], fp32, kind="ExternalOutput")

    mult = mybir.AluOpType.mult
    add = mybir.AluOpType.add
    relu = mybir.ActivationFunctionType.Relu
    copyf = mybir.ActivationFunctionType.Copy

    with ExitStack() as ctx:
        tc = ctx.enter_context(tile.TileContext(nc))
        lio = ctx.enter_context(tc.tile_pool(name="lio", bufs=4))
        gio = ctx.enter_context(tc.tile_pool(name="gio", bufs=4))
        gcv = ctx.enter_context(tc.tile_pool(name="gcv", bufs=4))
        prp = ctx.enter_context(tc.tile_pool(name="prp", bufs=6))
        psp = ctx.enter_context(tc.psum_pool(name="psp", bufs=4))
        hxp = ctx.enter_context(tc.tile_pool(name="hxp", bufs=4))
        acc = ctx.enter_context(tc.tile_pool(name="acc", bufs=1))

        ones_sb = acc.tile([P, 2 * P], bf16, tag="ones")
        nc.sync.dma_start(out=ones_sb[:], in_=onesc[:])
        pos = ones_sb[:, 0:P]
        neg = ones_sb[:, P:2 * P]

        hsum = acc.tile([P, 2 * NCH], fp32, tag="hsum")

        for k in range(NCH):
            li = lio.tile([P, A, CH], bf16, tag="li")
            nc.sync.dma_start(out=li[:], in_=img_loc[k])
            la = lio.tile([P, A, CH], bf16, tag="la")
            nc.sync.dma_start(out=la[:], in_=aud_loc[k])
            gi = gio.tile([P, A, CH], bf16, tag="gi")
            nc.sync.dma_start(out=gi[:], in_=img_imp[k])
            ga = gio.tile([P, A, CH], bf16, tag="ga")
            nc.sync.dma_start(out=ga[:], in_=aud_imp[k])

            prA = prp.tile([P, A, CH], bf16, tag="prA")
            nc.vector.tensor_tensor(out=prA[:], in0=li[:], in1=la[:], op=mult)
            prI = prp.tile([P, A, CH], bf16, tag="prI")
            nc.vector.tensor_tensor(out=prI[:], in0=gi[:], in1=la[:], op=mult)
            prU = prp.tile([P, A, CH], bf16, tag="prU")
            nc.vector.tensor_tensor(out=prU[:], in0=li[:], in1=ga[:], op=mult)

            px = psp.tile([P, CH], fp32, tag="px")
            py = psp.tile([P, CH], fp32, tag="py")
            for a in range(A):
                nc.tensor.matmul(px[:], pos, prI[:, a], start=(a == 0),
                                 stop=False, skip_group_check=True)
            for a in range(A):
                nc.tensor.matmul(py[:], pos, prU[:, a], start=(a == 0),
                                 stop=False, skip_group_check=True)
            for a in range(A):
                nc.tensor.matmul(px[:], neg, prA[:, a], start=False,
                                 stop=(a == A - 1), skip_group_check=True)
            for a in range(A):
                nc.tensor.matmul(py[:], neg, prA[:, a], start=False,
                                 stop=(a == A - 1), skip_group_check=True)

            hx = hxp.tile([P, CH], bf16, tag="hx")
            nc.scalar.activation(out=hx[:], in_=px[:], func=relu, bias=1.0,
                                 scale=1.0, accum_out=hsum[:, 2 * k:2 * k + 1])
            hy = hxp.tile([P, CH], bf16, tag="hy")
            nc.scalar.activation(out=hy[:], in_=py[:], func=relu, bias=1.0,
                                 scale=1.0,
                                 accum_out=hsum[:, 2 * k + 1:2 * k + 2])

        psum_t = acc.tile([P, 1], fp32, tag="psum")
        nc.vector.tensor_reduce(
            out=psum_t[:], in_=hsum[:], axis=mybir.AxisListType.X, op=add,
        )
        nc.sync.dma_start(out=partial[:], in_=psum_t[:])

    nc.compile()
    return nc


def _get_nc():
    if "nc" not in _CACHE:
        _CACHE["nc"] = _build_nc()
    return _CACHE["nc"]


def _block(xt):
    """[D, SHARD] -> [NCH, P, A, CH]: per (chunk, partition) contiguous."""
    return np.ascontiguousarray(
        xt.reshape(A, P, NCH, CH).transpose(2, 1, 0, 3))


def make_in_maps(image_outputs, audio_outputs, I_imp_ind, A_imp_ind):
    import ml_dtypes

    bf16 = np.dtype(ml_dtypes.bfloat16)
    fp8 = np.dtype(ml_dtypes.float8_e4m3fn)
    img = np.asarray(image_outputs, dtype=np.float32)
    aud = np.asarray(audio_outputs, dtype=np.float32)
    I_imp = np.asarray(I_imp_ind).astype(np.int64)
    A_imp = np.asarray(A_imp_ind).astype(np.int64)
    ones = np.concatenate(
        [np.ones((P, P), np.float32), -np.ones((P, P), np.float32)],
        axis=1).astype(bf16)
    in_maps = []
    for c in range(NCORES):
        base = c * SHARD
        sl = slice(base, base + SHARD)
        in_maps.append({
            "img_loc": _block(img[sl].T.astype(bf16)),
            "aud_loc": _block(aud[sl].T.astype(bf16)),
            "img_imp": _block(img[I_imp[sl]].T.astype(bf16)),
            "aud_imp": _block(aud[A_imp[sl]].T.astype(bf16)),
            "onesc": ones,
        })
    return in_maps


def kernel(image_outputs, audio_outputs, I_imp_ind, A_imp_ind):
    from concourse import bass_utils

    nc = _get_nc()
    in_maps = make_in_maps(image_outputs, audio_outputs, I_imp_ind, A_imp_ind)
    res = bass_utils.run_bass_kernel_spmd(nc, in_maps, list(range(NCORES))).results
    # every PSUM partition holds identical broadcast sums -> use row 0 only
    total = sum(float(r["partial"][0, 0]) for r in res)
    return np.float32(total / N)
